# revision 54
# baseline (speedup 1.0000x reference)
"""Trainium2 Bass kernel for nn_EnhancedHBitLinear (optimized v2).

Computation (per reference.py):
  x [2, 4096, 4096] -> flatten tokens T=8192
  xh = FWHT_4096(x) / 64 * had_scale + had_shift
  gamma[t] = max|xh[t,:]| + 1e-5 ; q = round(xh * 7/gamma)  (int4 levels)
  wscale = mean|W| + 1e-5 ; tern = clip(round(W/wscale), -1, 1)
  out[t,o] = sum_i q[t,i]*tern[o,i] * (gamma[t]/7) * wscale

Sharding: Megatron column-parallel. weight split into 8 shards of 2048 output
features; x / had_* replicated. Each core runs the full activation pipeline
+ its out-column shard; host concatenates shards.

v2 speedups over the first working version (~6.5 ms device time):
  - F1/F2 FWHT matmuls run as float32r (1 cyc/row vs fp32's 4; ~13-bit
    mantissa, verified on HW: adds ~0.006 rel err, gate is 2e-2)
  - main matmul: fp8e4 DoubleRow perf mode (0.5 cyc/row, exact for ints)
  - ternarized weights stay resident in SBUF (no per-group DRAM reload)
  - corner turn = in-place DVE 32x32 stream-transpose (zero DMA; the
    old per-chunk SBUF->SBUF DMA shuffle cost ~2.2us fixed per dma_start
    and serialized the HWDGE) + a Pool-engine f32r-laundering copy
  - per-token quant scale broadcast via PE (no DRAM round trip)
  - output stored bf16, upcast on host (halves store traffic)
  - work spread across all five engines: PE ~11us/blk (F1/F2/main mm),
    Act (F1 evac + 3/4 of F2 evac), DVE (stream transpose, rmax, rest of
    F2 evac), Pool (f32r copy + quant mult/round), SP (x load, out store)
"""

import math
import sys

import numpy as np

sys.path.insert(0, "/opt/trn_rl_repo")

import concourse.bass as bass
import concourse.bass_isa as bass_isa
import concourse.mybir as mybir
import concourse.tile as tile
from concourse import library_config
from concourse.bass_utils import run_bass_kernel_spmd

F32 = mybir.dt.float32
F32R = mybir.dt.float32r
BF16 = mybir.dt.bfloat16
FP8 = mybir.dt.float8e4

IN = 4096
OUT = 16384
N_CORES = 8
OSH = OUT // N_CORES  # 2048 out features per core
T_FULL = 8192

CH = 32   # feature chunks of 128 (IN/128)
LOHI = 32
LO4 = 4
EPS = 1e-5
ACT_QB = 7.0
MAGIC = 12582912.0  # 1.5 * 2**23 : v+M-M == round-half-even(v) for |v| < 2**22


def _hadamard(n):
    h = np.array([[1.0]], dtype=np.float32)
    while h.shape[0] < n:
        h = np.block([[h, h], [h, -h]])
    return h


def host_consts():
    """H128p: F1 stationary with output partitions permuted so partition
    m = lo4*32+lohi holds FWHT-low-bits index lo = lohi*4+lo4 (makes the
    shuffle DMA source slices partition-contiguous).
    S2: F2 stationary. S2[k=lo4p*32+cp, m=lo4o*32+co] = (lo4p==lo4o)*H32[co,cp].
    perm: flat feature permutation of the pipeline output:
    j = kk*128 + p2 (k-chunk kk=lohi, partition p2=lo4*32+c) -> original i."""
    H128 = _hadamard(128)
    m = np.arange(128)
    lo_of_m = (m % 32) * 4 + m // 32
    H128p = H128[:, lo_of_m].astype(np.float32).copy()

    H32 = _hadamard(32)
    # z2 partition packing is k = f*32 + c (f = lo4 major) so the corner
    # turn is a DVE 32x32 stream-transpose; S2[k=(f,cp), m=(c',f)] per-f block
    S2 = np.zeros((128, 128), dtype=np.float32)
    for f in range(LO4):
        for cp in range(32):
            for c in range(32):
                S2[f * 32 + cp, c * 4 + f] = H32[c, cp]
    perm = np.zeros(IN, dtype=np.int64)
    for kk in range(CH):
        for p2 in range(128):
            c = p2 // 4
            lo4 = p2 % 4
            perm[kk * 128 + p2] = c * 128 + kk * 4 + lo4
    return H128p, S2, perm


def build_program(n_cores=N_CORES, T=T_FULL, osh=OSH, t_blk=128,
                  debug=False, use_collective=True, out_bf16=True):
    """Build the single SPMD Bass program (identical on all cores)."""
    assert T % t_blk == 0
    nblk = T // t_blk
    n_ob = 4                     # psum banks for main matmul
    obw = osh // n_ob            # 512 out cols per bank

    nc = bass.Bass("TRN2", target_bir_lowering=False, debug=debug,
                   num_devices=n_cores)

    xt_d = nc.dram_tensor("xt", [IN, T], F32R, kind="ExternalInput")
    wt_d = nc.dram_tensor("wt", [IN, osh], F32, kind="ExternalInput")
    hs_d = nc.dram_tensor("hs2", [128, LOHI], F32, kind="ExternalInput")
    hb_d = nc.dram_tensor("hb2", [128, LOHI], F32, kind="ExternalInput")
    h1_d = nc.dram_tensor("h128p", [128, 128], F32R, kind="ExternalInput")
    s2_d = nc.dram_tensor("s2", [128, 128], F32R, kind="ExternalInput")
    id_d = nc.dram_tensor("id128", [128, 128], F32, kind="ExternalInput")
    out_dt = BF16 if out_bf16 else F32
    out_d = nc.dram_tensor("out", [T, osh], out_dt, kind="ExternalOutput")

    AL = mybir.AluOpType
    AF = mybir.ActivationFunctionType
    DR = mybir.MatmulPerfMode.DoubleRow

    with tile.TileContext(nc) as tc:
        from contextlib import ExitStack
        with ExitStack() as ctx:
            singles = ctx.enter_context(tc.tile_pool(name="singles", bufs=1))
            dramp = ctx.enter_context(
                tc.tile_pool(name="dramp", bufs=1, space=bass.MemorySpace.DRAM))

            cc_in = dramp.tile([1, 1], F32)
            cc_out = dramp.tile([1, 1], F32)

            # constants
            h1 = singles.tile([128, 128], F32R)
            nc.sync.dma_start(out=h1, in_=h1_d[:, :])
            s2 = singles.tile([128, 128], F32R)
            nc.sync.dma_start(out=s2, in_=s2_d[:, :])
            hs2 = singles.tile([128, LOHI], F32)
            nc.sync.dma_start(out=hs2, in_=hs_d[:, :])
            hb2 = singles.tile([128, LOHI], F32)
            nc.sync.dma_start(out=hb2, in_=hb_d[:, :])
            id128 = singles.tile([128, 128], F32)
            nc.sync.dma_start(out=id128, in_=id_d[:, :])
            ones128 = singles.tile([128, 1], F32)
            nc.vector.memset(ones128, 1.0)
            onesrow = singles.tile([1, 128], F32)
            nc.vector.memset(onesrow, 1.0)

            # small psum scratch: one bank shared by slices (weight-sum,
            # gamma transpose, quant-scale broadcast)
            psg = ctx.enter_context(
                tc.tile_pool(name="psg", bufs=1, space="PSUM"))
            smallp = psg.tile([128, 512], F32, tag="small", name="smallp")

            # ---------------- weight prep ----------------
            # wq2 tiles stay resident in SBUF: 16 pair-tiles [128, 2, osh] fp8
            wqp = ctx.enter_context(tc.tile_pool(name="wqp", bufs=1))
            wq2 = [wqp.tile([128, 2, osh], FP8, tag=f"wq{k}", name=f"wq{k}")
                   for k in range(CH // 2)]

            scl128 = singles.tile([128, 1], F32)   # wscale bcast
            thr128 = singles.tile([128, 1], F32)   # +wscale/2
            nthr128 = singles.tile([128, 1], F32)  # -wscale/2
            ws7 = singles.tile([128, 1], F32)      # wscale/7
            with tc.tile_pool(name="wprep", bufs=2) as wp, \
                 tc.tile_pool(name="wacc", bufs=1) as wa:
                accs = wa.tile([128, CH], F32)
                for kk in range(CH):
                    wf = wp.tile([128, osh], F32, tag="wf")
                    nc.gpsimd.dma_start(out=wf,
                                        in_=wt_d[kk * 128:(kk + 1) * 128, :])
                    nc.vector.tensor_reduce(
                        out=accs[:, kk:kk + 1], in_=wf,
                        axis=mybir.AxisListType.X, op=AL.add,
                        apply_absolute_value=True)
                tot128 = wa.tile([128, 1], F32)
                nc.vector.tensor_reduce(out=tot128, in_=accs,
                                        axis=mybir.AxisListType.X, op=AL.add)
                tot1p = smallp[0:1, 384:385]
                nc.tensor.matmul(tot1p, lhsT=ones128, rhs=tot128,
                                 start=True, stop=True)
                tot1 = wa.tile([1, 1], F32)
                nc.scalar.copy(tot1, tot1p)
                nc.sync.dma_start(out=cc_in[:, :], in_=tot1)
                if use_collective:
                    nc.gpsimd.collective_compute(
                        "AllReduce", AL.add,
                        replica_groups=[list(range(n_cores))],
                        ins=[cc_in[:, :]], outs=[cc_out[:, :]])
                else:
                    nc.sync.dma_start(out=cc_out[:, :], in_=cc_in[:, :])
                totg = wa.tile([128, 1], F32)
                cc_bcast = bass.AP(tensor=cc_out.tensor, offset=cc_out.offset,
                                   ap=[[0, 128], [1, 1]])
                nc.gpsimd.dma_start(out=totg, in_=cc_bcast)
                nc.vector.tensor_scalar(
                    out=scl128, in0=totg, scalar1=1.0 / (IN * osh * n_cores),
                    scalar2=EPS, op0=AL.mult, op1=AL.add)
                nc.vector.tensor_scalar_mul(out=thr128, in0=scl128, scalar1=0.5)
                nc.vector.tensor_scalar_mul(out=nthr128, in0=scl128,
                                            scalar1=-0.5)
                nc.vector.tensor_scalar_mul(out=ws7, in0=scl128,
                                            scalar1=1.0 / 7.0)
                for kk in range(CH):
                    wf = wp.tile([128, osh], F32, tag="wf")
                    nc.gpsimd.dma_start(out=wf,
                                        in_=wt_d[kk * 128:(kk + 1) * 128, :])
                    g = wp.tile([128, osh], F32, tag="g")
                    nc.vector.tensor_scalar(out=g, in0=wf, scalar1=thr128,
                                            scalar2=None, op0=AL.is_gt)
                    l = wp.tile([128, osh], F32, tag="l")
                    nc.gpsimd.tensor_scalar(out=l, in0=wf, scalar1=nthr128,
                                            scalar2=None, op0=AL.is_lt)
                    nc.vector.tensor_tensor(out=wq2[kk // 2][:, kk % 2, :],
                                            in0=g, in1=l, op=AL.subtract)

            # ---------------- main pipeline ----------------
            xp = ctx.enter_context(tc.tile_pool(name="xp", bufs=2))
            zep = ctx.enter_context(tc.tile_pool(name="zep", bufs=2))
            z2p = ctx.enter_context(tc.tile_pool(name="z2p", bufs=2))
            y2p = ctx.enter_context(tc.tile_pool(name="y2p", bufs=2))
            qp = ctx.enter_context(tc.tile_pool(name="qp", bufs=2))
            op_ = ctx.enter_context(tc.tile_pool(name="op", bufs=2))
            gp = ctx.enter_context(tc.tile_pool(name="gp", bufs=2))
            psf1 = ctx.enter_context(
                tc.tile_pool(name="psf1", bufs=1, space="PSUM"))
            psf2 = ctx.enter_context(
                tc.tile_pool(name="psf2", bufs=2, space="PSUM"))
            psm = ctx.enter_context(
                tc.tile_pool(name="psm", bufs=1, space="PSUM"))

            for blk in range(nblk):
                t0 = blk * t_blk
                # ---- x load (two halves): [128 p, 16 c, t] ----
                xs = []
                for xh in range(2):
                    x_t = xp.tile([128, CH // 2, t_blk], F32R, tag="x",
                                  name=f"x_{blk}_{xh}")
                    nc.sync.dma_start(
                        out=x_t,
                        in_=xt_d[:, t0:t0 + t_blk].rearrange(
                            "(c p) t -> p c t", p=128)[
                                :, xh * (CH // 2):(xh + 1) * (CH // 2), :])
                    xs.append(x_t)
                # ---- F1: H128 per chunk; 2 matmuls x 2 chunks per bank ----
                zE = zep.tile([128, CH, t_blk], F32, tag="ze")
                for c4 in range(CH // 4):
                    x_t = xs[c4 // 4]
                    xc = 4 * c4 - (c4 // 4) * (CH // 2)
                    pf1 = psf1.tile([128, 2, 2 * t_blk], F32, tag="pf1")
                    for i in range(2):
                        nc.tensor.matmul(
                            pf1[:, i, :],
                            lhsT=h1,
                            rhs=x_t[:, xc + 2 * i:xc + 2 * i + 2, :],
                            start=True, stop=True)
                    nc.scalar.copy(zE[:, 4 * c4:4 * c4 + 4, :], pf1)
                # ---- corner turn: in-place DVE 32x32 stream-transpose ----
                # zE[(f h), c, t] -> zE[(f c), h, t]: per partition-group f
                # and token t, transpose the 32x32 (h x c) square in place.
                # Free AP order (t, inner-32) makes each 32-window the h/c
                # dim. Pool then copies to z2 rounding to f32r (the BIR
                # verifier requires an explicit f32r-producing instruction
                # ahead of an f32r matmul; StreamTranspose is fp32-only).
                nc.vector.transpose(
                    out=zE.rearrange("m c t -> m t c"),
                    in_=zE.rearrange("m c t -> m t c"))
                z2 = z2p.tile([128, LOHI, t_blk], F32R, tag="z2")
                # ---- F2 + evac (scale/bias per lohi) ----
                # y2 is (t, h)-major so the gamma reduce runs contiguous
                y2 = y2p.tile([128, t_blk, LOHI], F32, tag="y2")
                lohi_per_mm = 512 // t_blk
                rmax = gp.tile([128, t_blk], F32, tag="rmax")
                rmax_a = gp.tile([128, t_blk], F32, tag="rmax_a")
                n_s = LOHI // lohi_per_mm
                for s in range(n_s):
                    sl = slice(s * lohi_per_mm, (s + 1) * lohi_per_mm)
                    # f32r-launder this slice of the turned zE on Pool
                    nc.gpsimd.tensor_copy(out=z2[:, sl, :], in_=zE[:, sl, :])
                    pf2 = psf2.tile([128, 512], F32, tag="pf2")
                    nc.tensor.matmul(
                        pf2, lhsT=s2,
                        rhs=z2[:, sl, :],
                        start=True, stop=True)
                    for j in range(lohi_per_mm):
                        lohi = s * lohi_per_mm + j
                        if j < 3:
                            nc.scalar.activation(
                                out=y2[:, :, lohi],
                                in_=pf2[:, j * t_blk:(j + 1) * t_blk],
                                func=AF.Identity,
                                scale=hs2[:, lohi:lohi + 1],
                                bias=hb2[:, lohi:lohi + 1])
                        else:
                            nc.vector.tensor_scalar(
                                out=y2[:, :, lohi],
                                in0=pf2[:, j * t_blk:(j + 1) * t_blk],
                                scalar1=hs2[:, lohi:lohi + 1],
                                scalar2=hb2[:, lohi:lohi + 1],
                                op0=AL.mult, op1=AL.add)
                    if s == n_s // 2 - 1:
                        # first-half abs-max starts while second half runs
                        nc.vector.tensor_reduce(
                            out=rmax_a, in_=y2[:, :, 0:LOHI // 2],
                            axis=mybir.AxisListType.X, op=AL.max,
                            apply_absolute_value=True)
                # ---- gamma ----
                nc.vector.tensor_reduce(
                    out=rmax, in_=y2[:, :, LOHI // 2:],
                    axis=mybir.AxisListType.X, op=AL.max,
                    apply_absolute_value=True)
                nc.vector.tensor_tensor(out=rmax, in0=rmax, in1=rmax_a,
                                        op=AL.max)
                rmT = smallp[:, 0:t_blk]
                nc.tensor.transpose(rmT[0:t_blk, :], rmax, id128)
                gam = gp.tile([t_blk, 1], F32, tag="gam")
                nc.vector.tensor_reduce(
                    out=gam, in_=rmT[0:t_blk, :],
                    axis=mybir.AxisListType.X, op=AL.max)
                nc.vector.tensor_scalar_add(out=gam, in0=gam, scalar1=EPS)
                so = gp.tile([t_blk, 1], F32, tag="so")
                nc.vector.tensor_tensor(out=so, in0=gam, in1=ws7, op=AL.mult)
                sbc = gp.tile([t_blk, 1], F32, tag="sbc")
                nc.vector.reciprocal(out=sbc, in_=gam)
                nc.vector.tensor_scalar_mul(out=sbc, in0=sbc, scalar1=ACT_QB)
                # broadcast sbc over partitions via PE
                sbT = smallp[0:1, 128:128 + t_blk]
                nc.tensor.transpose(sbT, sbc, id128)
                sbrow = gp.tile([1, t_blk], F32, tag="sbrow")
                nc.scalar.copy(sbrow, sbT)
                sbb = smallp[:, 256:256 + t_blk]
                nc.tensor.matmul(sbb, lhsT=onesrow, rhs=sbrow,
                                 start=True, stop=True)
                sbbS = gp.tile([128, t_blk], F32, tag="sbbs")
                nc.vector.tensor_copy(out=sbbS, in_=sbb)
                # ---- quant (Pool): y2 *= sbb ; round; cast fp8 ----
                # two h-halves so the main matmul can start on early chunks
                qb = qp.tile([128, CH, t_blk], FP8, tag="q")
                for qh in range(2):
                    hsl = slice(qh * (LOHI // 2), (qh + 1) * (LOHI // 2))
                    a1, a2 = bass.broadcast_tensor_aps(
                        y2[:, :, hsl],
                        sbbS.rearrange("p (t o) -> p t o", o=1))
                    nc.gpsimd.tensor_tensor(out=y2[:, :, hsl], in0=a1,
                                            in1=a2, op=AL.mult)
                    nc.gpsimd.tensor_scalar(
                        out=qb[:, hsl, :].rearrange("p c t -> p t c"),
                        in0=y2[:, :, hsl], scalar1=MAGIC,
                        scalar2=MAGIC, op0=AL.add, op1=AL.subtract)
                # ---- main matmul: fp8 DoubleRow, q stationary, 4 banks ----
                ot = op_.tile([128, osh], out_dt, tag="ot")
                psums = [psm.tile([128, obw], F32, tag=f"pm{ob}",
                                  name=f"pm{ob}")
                         for ob in range(n_ob)]
                for k2 in range(CH // 2):
                    lhs = qb[:, 2 * k2:2 * k2 + 2, :]
                    for ob in range(n_ob):
                        nc.tensor.matmul(
                            psums[ob], lhsT=lhs,
                            rhs=wq2[k2][:, :, ob * obw:(ob + 1) * obw],
                            start=(k2 == 0), stop=(k2 == CH // 2 - 1),
                            perf_mode=DR)
                # evac split across Act and DVE (scale = gamma*wscale/7)
                for ob in range(2):
                    nc.scalar.activation(
                        out=ot[:, ob * obw:(ob + 1) * obw],
                        in_=psums[ob], func=AF.Copy, scale=so)
                for ob in range(2, n_ob):
                    nc.vector.tensor_scalar_mul(
                        out=ot[:, ob * obw:(ob + 1) * obw],
                        in0=psums[ob], scalar1=so)
                nc.sync.dma_start(out=out_d[t0:t0 + t_blk, :], in_=ot)

    return nc


def _split_multi_waits(nc):
    """walrus's CTRL encoder fits one sem-wait per instruction; Tile can emit
    several (e.g. the kernel-tail drain). Hoist extras onto standalone
    InstEventSemaphore carriers inserted just before the instruction."""
    import copy

    m = nc.m
    new_module = copy.replace(m, functions=[])
    ctr = 0
    for function in m.functions:
        new_function = copy.replace(function, blocks=[])
        new_function.set_allocations_from_list(function.allocations)
        for block in function.blocks:
            new_insts = []
            for inst in block.instructions:
                si = inst.sync_info
                ow = list(si.on_wait) if si is not None and si.on_wait else []
                if len(ow) > 1:
                    for w in ow[:-1]:
                        ctr += 1
                        new_insts.append(mybir.InstEventSemaphore(
                            name=f"I-wsplit-{ctr}",
                            engine=inst.engine,
                            ins=[], outs=[],
                            sync_info=mybir.SyncInfo(on_wait=[w],
                                                     on_update=[])))
                    inst = copy.replace(
                        inst,
                        sync_info=mybir.SyncInfo(on_wait=[ow[-1]],
                                                 on_update=si.on_update))
                new_insts.append(inst)
            new_block = copy.replace(block, instructions=new_insts)
            new_function.blocks.append(new_block)
        new_module.functions.append(new_function)
    nc.m = new_module
    return ctr


def host_prep(x, weight, had_scale, had_shift, n_cores=N_CORES, osh=None):
    """Shard + re-layout inputs for the SPMD program. Layout prep only."""
    T = int(np.prod(x.shape[:-1]))
    osh = osh or weight.shape[0] // n_cores
    H128p, S2, perm = host_consts()
    xt = np.ascontiguousarray(x.reshape(T, IN).T)  # [4096, T]
    id128 = np.eye(128, dtype=np.float32)
    # fold the 1/64 FWHT normalization into the had_scale operand
    hs2 = np.ascontiguousarray(
        (had_scale[perm] / 64.0).reshape(CH, 128).T)  # [128(p2), 32(kk)]
    hb2 = np.ascontiguousarray(had_shift[perm].reshape(CH, 128).T)
    in_maps = []
    for core in range(n_cores):
        wsh = weight[core * osh:(core + 1) * osh, :]  # [osh, IN]
        wt = np.ascontiguousarray(wsh[:, perm].T)     # [IN(perm j), osh]
        in_maps.append({
            "xt": xt, "wt": wt, "hs2": hs2, "hb2": hb2,
            "h128p": H128p, "s2": S2, "id128": id128,
        })
    return in_maps


_PROGRAM_CACHE = {}


def _get_program(key, **kwargs):
    if key not in _PROGRAM_CACHE:
        nc = build_program(**kwargs)
        _split_multi_waits(nc)
        _PROGRAM_CACHE[key] = nc
    return _PROGRAM_CACHE[key]


def _timed_pjrt(nc, in_maps, n_cores, ks=(1, 17), reps=3):
    """Run via PJRT shard_map with device-resident inputs.

    Timing: dispatches pipeline asynchronously, so wall(K executions) =
    dispatch_overhead + K * device_time. The slope between two batch sizes
    isolates the marginal device execution time from the (~75ms here)
    axon-tunnel dispatch latency. Returns (results, times_dict).
    """
    import time

    import jax
    import jax.numpy as jnp
    from jax.sharding import Mesh, NamedSharding, PartitionSpec
    from jax.experimental.shard_map import shard_map

    from concourse import bass2jax, mybir as mb

    bass2jax.install_neuronx_cc_hook()

    partition_name = (nc.partition_id_tensor.name
                      if nc.partition_id_tensor else None)
    in_names, out_names, out_avals, zero_outs = [], [], [], []
    for alloc in nc.m.functions[0].allocations:
        if not isinstance(alloc, mb.MemoryLocationSet):
            continue
        name = alloc.memorylocations[0].name
        if alloc.kind == "ExternalInput":
            if name != partition_name:
                in_names.append(name)
        elif alloc.kind == "ExternalOutput":
            out_names.append(name)
            shape = tuple(alloc.tensor_shape)
            dtype = mb.dt.np(alloc.dtype)
            out_avals.append(jax.core.ShapedArray(shape, dtype))
            zero_outs.append(np.zeros(shape, dtype))
    n_params = len(in_names)
    all_in_names = list(in_names) + list(out_names)
    if partition_name is not None:
        all_in_names.append(partition_name)

    def _body(*args):
        operands = list(args)
        if partition_name is not None:
            operands.append(bass2jax.partition_id_tensor())
        outs = bass2jax._bass_exec_p.bind(
            *operands,
            out_avals=tuple(out_avals),
            in_names=tuple(all_in_names),
            out_names=tuple(out_names),
            lowering_input_output_aliases=(),
            sim_require_finite=True,
            sim_require_nnan=True,
            nc=nc,
        )
        return tuple(outs)

    devices = jax.devices()[:n_cores]
    mesh = Mesh(np.asarray(devices), ("core",))
    spec = NamedSharding(mesh, PartitionSpec("core"))
    n_outs = len(out_names)
    donate = tuple(range(n_params, n_params + n_outs))
    sharded = jax.jit(
        shard_map(_body, mesh=mesh,
                  in_specs=(PartitionSpec("core"),) * (n_params + n_outs),
                  out_specs=(PartitionSpec("core"),) * n_outs,
                  check_rep=False),
        donate_argnums=donate, keep_unused=True)

    concat_in = [
        np.concatenate([np.asarray(in_maps[c][nm]) for c in range(n_cores)],
                       axis=0)
        for nm in in_names
    ]
    dev_in = [jax.device_put(a, spec) for a in concat_in]
    zero_shapes = [(n_cores * z.shape[0], *z.shape[1:]) for z in zero_outs]
    make_zeros = jax.jit(
        lambda: tuple(jnp.zeros(s, z.dtype)
                      for s, z in zip(zero_shapes, zero_outs)),
        out_shardings=(spec,) * n_outs)

    out = sharded(*dev_in, *make_zeros())  # warmup/compile
    jax.block_until_ready(out)

    times = {}
    for k in ks:
        times[k] = []
        for _ in range(reps):
            zs = [make_zeros() for _ in range(k)]
            jax.block_until_ready(zs)
            t0 = time.perf_counter()
            for i in range(k):
                out = sharded(*dev_in, *zs[i])
            jax.block_until_ready(out)
            times[k].append(time.perf_counter() - t0)
    results = [
        {nm: np.asarray(out[i]).reshape(n_cores, *out_avals[i].shape)[c]
         for i, nm in enumerate(out_names)}
        for c in range(n_cores)
    ]
    return results, times


def run(x, weight, had_scale, had_shift, trace=False, **trace_kwargs):
    x = np.asarray(x, dtype=np.float32)
    weight = np.asarray(weight, dtype=np.float32)
    had_scale = np.asarray(had_scale, dtype=np.float32)
    had_shift = np.asarray(had_shift, dtype=np.float32)
    batch_shape = x.shape[:-1]
    T = int(np.prod(batch_shape))
    nc = _get_program(("full", T), T=T)
    in_maps = host_prep(x, weight, had_scale, had_shift)
    if trace:
        results, times = _timed_pjrt(nc, in_maps, N_CORES)
    else:
        res = run_bass_kernel_spmd(nc, in_maps, core_ids=list(range(N_CORES)))
        results, times = res.results, None
    shards = [np.asarray(results[c]["out"], dtype=np.float32)
              for c in range(N_CORES)]
    out = np.concatenate(shards, axis=1).reshape(*batch_shape, OUT)
    return out, times


def kernel(**inputs):
    out, _ = run(inputs["x"], inputs["weight"], inputs["had_scale"],
                 inputs["had_shift"])
    return out


if __name__ == "__main__":
    nc = build_program(T=256)
    print("build ok")


# revision 57
# speedup vs baseline: 1.0166x; 1.0166x over previous
"""Trainium2 Bass kernel for nn_EnhancedHBitLinear (optimized v2).

Computation (per reference.py):
  x [2, 4096, 4096] -> flatten tokens T=8192
  xh = FWHT_4096(x) / 64 * had_scale + had_shift
  gamma[t] = max|xh[t,:]| + 1e-5 ; q = round(xh * 7/gamma)  (int4 levels)
  wscale = mean|W| + 1e-5 ; tern = clip(round(W/wscale), -1, 1)
  out[t,o] = sum_i q[t,i]*tern[o,i] * (gamma[t]/7) * wscale

Sharding: Megatron column-parallel. weight split into 8 shards of 2048 output
features; x / had_* replicated. Each core runs the full activation pipeline
+ its out-column shard; host concatenates shards.

v2 speedups over the first working version (~6.5 ms device time):
  - F1/F2 FWHT matmuls run as float32r (1 cyc/row vs fp32's 4; ~13-bit
    mantissa, verified on HW: adds ~0.006 rel err, gate is 2e-2)
  - main matmul: fp8e4 DoubleRow perf mode (0.5 cyc/row, exact for ints)
  - ternarized weights stay resident in SBUF (no per-group DRAM reload)
  - corner turn = in-place DVE 32x32 stream-transpose (zero DMA; the
    old per-chunk SBUF->SBUF DMA shuffle cost ~2.2us fixed per dma_start
    and serialized the HWDGE) + a Pool-engine f32r-laundering copy
  - per-token quant scale broadcast via PE (no DRAM round trip)
  - output stored bf16, upcast on host (halves store traffic)
  - work spread across all five engines: PE ~11us/blk (F1/F2/main mm),
    Act (F1 evac + 3/4 of F2 evac), DVE (stream transpose, rmax, rest of
    F2 evac), Pool (f32r copy + quant mult/round), SP (x load, out store)
"""

import math
import sys

import numpy as np

sys.path.insert(0, "/opt/trn_rl_repo")

import concourse.bass as bass
import concourse.bass_isa as bass_isa
import concourse.mybir as mybir
import concourse.tile as tile
from concourse import library_config
from concourse.bass_utils import run_bass_kernel_spmd

F32 = mybir.dt.float32
F32R = mybir.dt.float32r
BF16 = mybir.dt.bfloat16
FP8 = mybir.dt.float8e4

IN = 4096
OUT = 16384
N_CORES = 8
OSH = OUT // N_CORES  # 2048 out features per core
T_FULL = 8192

CH = 32   # feature chunks of 128 (IN/128)
LOHI = 32
LO4 = 4
EPS = 1e-5
ACT_QB = 7.0
MAGIC = 12582912.0  # 1.5 * 2**23 : v+M-M == round-half-even(v) for |v| < 2**22


def _hadamard(n):
    h = np.array([[1.0]], dtype=np.float32)
    while h.shape[0] < n:
        h = np.block([[h, h], [h, -h]])
    return h


def host_consts():
    """H128p: F1 stationary with output partitions permuted so partition
    m = lo4*32+lohi holds FWHT-low-bits index lo = lohi*4+lo4 (makes the
    shuffle DMA source slices partition-contiguous).
    S2: F2 stationary. S2[k=lo4p*32+cp, m=lo4o*32+co] = (lo4p==lo4o)*H32[co,cp].
    perm: flat feature permutation of the pipeline output:
    j = kk*128 + p2 (k-chunk kk=lohi, partition p2=lo4*32+c) -> original i."""
    H128 = _hadamard(128)
    m = np.arange(128)
    lo_of_m = (m % 32) * 4 + m // 32
    H128p = H128[:, lo_of_m].astype(np.float32).copy()

    H32 = _hadamard(32)
    # z2 partition packing is k = f*32 + c (f = lo4 major) so the corner
    # turn is a DVE 32x32 stream-transpose; S2[k=(f,cp), m=(c',f)] per-f block
    S2 = np.zeros((128, 128), dtype=np.float32)
    for f in range(LO4):
        for cp in range(32):
            for c in range(32):
                S2[f * 32 + cp, c * 4 + f] = H32[c, cp]
    perm = np.zeros(IN, dtype=np.int64)
    for kk in range(CH):
        for p2 in range(128):
            c = p2 // 4
            lo4 = p2 % 4
            perm[kk * 128 + p2] = c * 128 + kk * 4 + lo4
    return H128p, S2, perm


def build_program(n_cores=N_CORES, T=T_FULL, osh=OSH, t_blk=128,
                  debug=False, use_collective=True, out_bf16=True):
    """Build the single SPMD Bass program (identical on all cores)."""
    assert T % t_blk == 0
    nblk = T // t_blk
    n_ob = 4                     # psum banks for main matmul
    obw = osh // n_ob            # 512 out cols per bank

    nc = bass.Bass("TRN2", target_bir_lowering=False, debug=debug,
                   num_devices=n_cores)

    # x pre-tiled on host to the on-chip block layout [blk, p, c, t] so
    # each block load is one contiguous 2MB read (32x64KB descriptors
    # instead of 4096x512B row-gather descriptors)
    xt_d = nc.dram_tensor("xt", [T // t_blk, 128, CH, t_blk], F32R,
                          kind="ExternalInput")
    wt_d = nc.dram_tensor("wt", [IN, osh], F32, kind="ExternalInput")
    hs_d = nc.dram_tensor("hs2", [128, LOHI], F32, kind="ExternalInput")
    hb_d = nc.dram_tensor("hb2", [128, LOHI], F32, kind="ExternalInput")
    h1_d = nc.dram_tensor("h128p", [128, 128], F32R, kind="ExternalInput")
    s2_d = nc.dram_tensor("s2", [128, 128], F32R, kind="ExternalInput")
    id_d = nc.dram_tensor("id128", [128, 128], F32, kind="ExternalInput")
    out_dt = BF16 if out_bf16 else F32
    out_d = nc.dram_tensor("out", [T, osh], out_dt, kind="ExternalOutput")

    AL = mybir.AluOpType
    AF = mybir.ActivationFunctionType
    DR = mybir.MatmulPerfMode.DoubleRow

    with tile.TileContext(nc) as tc:
        from contextlib import ExitStack
        with ExitStack() as ctx:
            singles = ctx.enter_context(tc.tile_pool(name="singles", bufs=1))
            dramp = ctx.enter_context(
                tc.tile_pool(name="dramp", bufs=1, space=bass.MemorySpace.DRAM))

            cc_in = dramp.tile([1, 1], F32)
            cc_out = dramp.tile([1, 1], F32)

            # constants
            h1 = singles.tile([128, 128], F32R)
            nc.sync.dma_start(out=h1, in_=h1_d[:, :])
            s2 = singles.tile([128, 128], F32R)
            nc.sync.dma_start(out=s2, in_=s2_d[:, :])
            hs2 = singles.tile([128, LOHI], F32)
            nc.sync.dma_start(out=hs2, in_=hs_d[:, :])
            hb2 = singles.tile([128, LOHI], F32)
            nc.sync.dma_start(out=hb2, in_=hb_d[:, :])
            id128 = singles.tile([128, 128], F32)
            nc.sync.dma_start(out=id128, in_=id_d[:, :])
            ones128 = singles.tile([128, 1], F32)
            nc.vector.memset(ones128, 1.0)
            onesrow = singles.tile([1, 128], F32)
            nc.vector.memset(onesrow, 1.0)

            # small psum scratch: one bank shared by slices (weight-sum,
            # gamma transpose, quant-scale broadcast)
            psg = ctx.enter_context(
                tc.tile_pool(name="psg", bufs=1, space="PSUM"))
            smallp = psg.tile([128, 512], F32, tag="small", name="smallp")

            # ---------------- weight prep ----------------
            # wq2 tiles stay resident in SBUF: 16 pair-tiles [128, 2, osh] fp8
            wqp = ctx.enter_context(tc.tile_pool(name="wqp", bufs=1))
            wq2 = [wqp.tile([128, 2, osh], FP8, tag=f"wq{k}", name=f"wq{k}")
                   for k in range(CH // 2)]

            scl128 = singles.tile([128, 1], F32)   # wscale bcast
            thr128 = singles.tile([128, 1], F32)   # +wscale/2
            nthr128 = singles.tile([128, 1], F32)  # -wscale/2
            ws7 = singles.tile([128, 1], F32)      # wscale/7
            with tc.tile_pool(name="wprep", bufs=2) as wp, \
                 tc.tile_pool(name="wacc", bufs=1) as wa:
                accs = wa.tile([128, CH], F32)
                for kk in range(CH):
                    wf = wp.tile([128, osh], F32, tag="wf")
                    nc.gpsimd.dma_start(out=wf,
                                        in_=wt_d[kk * 128:(kk + 1) * 128, :])
                    nc.vector.tensor_reduce(
                        out=accs[:, kk:kk + 1], in_=wf,
                        axis=mybir.AxisListType.X, op=AL.add,
                        apply_absolute_value=True)
                tot128 = wa.tile([128, 1], F32)
                nc.vector.tensor_reduce(out=tot128, in_=accs,
                                        axis=mybir.AxisListType.X, op=AL.add)
                tot1p = smallp[0:1, 384:385]
                nc.tensor.matmul(tot1p, lhsT=ones128, rhs=tot128,
                                 start=True, stop=True)
                tot1 = wa.tile([1, 1], F32)
                nc.scalar.copy(tot1, tot1p)
                nc.sync.dma_start(out=cc_in[:, :], in_=tot1)
                if use_collective:
                    nc.gpsimd.collective_compute(
                        "AllReduce", AL.add,
                        replica_groups=[list(range(n_cores))],
                        ins=[cc_in[:, :]], outs=[cc_out[:, :]])
                else:
                    nc.sync.dma_start(out=cc_out[:, :], in_=cc_in[:, :])
                totg = wa.tile([128, 1], F32)
                cc_bcast = bass.AP(tensor=cc_out.tensor, offset=cc_out.offset,
                                   ap=[[0, 128], [1, 1]])
                nc.gpsimd.dma_start(out=totg, in_=cc_bcast)
                nc.vector.tensor_scalar(
                    out=scl128, in0=totg, scalar1=1.0 / (IN * osh * n_cores),
                    scalar2=EPS, op0=AL.mult, op1=AL.add)
                nc.vector.tensor_scalar_mul(out=thr128, in0=scl128, scalar1=0.5)
                nc.vector.tensor_scalar_mul(out=nthr128, in0=scl128,
                                            scalar1=-0.5)
                nc.vector.tensor_scalar_mul(out=ws7, in0=scl128,
                                            scalar1=1.0 / 7.0)
                for kk in range(CH):
                    wf = wp.tile([128, osh], F32, tag="wf")
                    nc.gpsimd.dma_start(out=wf,
                                        in_=wt_d[kk * 128:(kk + 1) * 128, :])
                    g = wp.tile([128, osh], F32, tag="g")
                    nc.vector.tensor_scalar(out=g, in0=wf, scalar1=thr128,
                                            scalar2=None, op0=AL.is_gt)
                    l = wp.tile([128, osh], F32, tag="l")
                    nc.gpsimd.tensor_scalar(out=l, in0=wf, scalar1=nthr128,
                                            scalar2=None, op0=AL.is_lt)
                    nc.vector.tensor_tensor(out=wq2[kk // 2][:, kk % 2, :],
                                            in0=g, in1=l, op=AL.subtract)

            # ---------------- main pipeline ----------------
            xp = ctx.enter_context(tc.tile_pool(name="xp", bufs=2))
            zep = ctx.enter_context(tc.tile_pool(name="zep", bufs=2))
            z2p = ctx.enter_context(tc.tile_pool(name="z2p", bufs=2))
            y2p = ctx.enter_context(tc.tile_pool(name="y2p", bufs=2))
            qp = ctx.enter_context(tc.tile_pool(name="qp", bufs=2))
            op_ = ctx.enter_context(tc.tile_pool(name="op", bufs=2))
            gp = ctx.enter_context(tc.tile_pool(name="gp", bufs=2))
            psf1 = ctx.enter_context(
                tc.tile_pool(name="psf1", bufs=1, space="PSUM"))
            psf2 = ctx.enter_context(
                tc.tile_pool(name="psf2", bufs=2, space="PSUM"))
            psm = ctx.enter_context(
                tc.tile_pool(name="psm", bufs=1, space="PSUM"))

            for blk in range(nblk):
                t0 = blk * t_blk
                # ---- x load (two halves): [128 p, 16 c, t] ----
                xs = []
                for xh in range(2):
                    x_t = xp.tile([128, CH // 2, t_blk], F32R, tag="x",
                                  name=f"x_{blk}_{xh}")
                    nc.sync.dma_start(
                        out=x_t,
                        in_=xt_d[blk, :,
                                 xh * (CH // 2):(xh + 1) * (CH // 2), :])
                    xs.append(x_t)
                # ---- F1: H128 per chunk; 2 matmuls x 2 chunks per bank ----
                zE = zep.tile([128, CH, t_blk], F32, tag="ze")
                for c4 in range(CH // 4):
                    x_t = xs[c4 // 4]
                    xc = 4 * c4 - (c4 // 4) * (CH // 2)
                    pf1 = psf1.tile([128, 2, 2 * t_blk], F32, tag="pf1")
                    for i in range(2):
                        nc.tensor.matmul(
                            pf1[:, i, :],
                            lhsT=h1,
                            rhs=x_t[:, xc + 2 * i:xc + 2 * i + 2, :],
                            start=True, stop=True)
                    nc.scalar.copy(zE[:, 4 * c4:4 * c4 + 4, :], pf1)
                # ---- corner turn: in-place DVE 32x32 stream-transpose ----
                # zE[(f h), c, t] -> zE[(f c), h, t]: per partition-group f
                # and token t, transpose the 32x32 (h x c) square in place.
                # Free AP order (t, inner-32) makes each 32-window the h/c
                # dim. Pool then copies to z2 rounding to f32r (the BIR
                # verifier requires an explicit f32r-producing instruction
                # ahead of an f32r matmul; StreamTranspose is fp32-only).
                nc.vector.transpose(
                    out=zE.rearrange("m c t -> m t c"),
                    in_=zE.rearrange("m c t -> m t c"))
                z2 = z2p.tile([128, LOHI, t_blk], F32R, tag="z2")
                # ---- F2 + evac (scale/bias per lohi) ----
                # y2 is (t, h)-major so the gamma reduce runs contiguous
                y2 = y2p.tile([128, t_blk, LOHI], F32, tag="y2")
                lohi_per_mm = 512 // t_blk
                rmax = gp.tile([128, t_blk], F32, tag="rmax")
                rmax_a = gp.tile([128, t_blk], F32, tag="rmax_a")
                n_s = LOHI // lohi_per_mm
                for s in range(n_s):
                    sl = slice(s * lohi_per_mm, (s + 1) * lohi_per_mm)
                    # f32r-launder this slice of the turned zE on Pool
                    nc.gpsimd.tensor_copy(out=z2[:, sl, :], in_=zE[:, sl, :])
                    pf2 = psf2.tile([128, 512], F32, tag="pf2")
                    nc.tensor.matmul(
                        pf2, lhsT=s2,
                        rhs=z2[:, sl, :],
                        start=True, stop=True)
                    for j in range(lohi_per_mm):
                        lohi = s * lohi_per_mm + j
                        if j < 3:
                            nc.scalar.activation(
                                out=y2[:, :, lohi],
                                in_=pf2[:, j * t_blk:(j + 1) * t_blk],
                                func=AF.Identity,
                                scale=hs2[:, lohi:lohi + 1],
                                bias=hb2[:, lohi:lohi + 1])
                        else:
                            nc.vector.tensor_scalar(
                                out=y2[:, :, lohi],
                                in0=pf2[:, j * t_blk:(j + 1) * t_blk],
                                scalar1=hs2[:, lohi:lohi + 1],
                                scalar2=hb2[:, lohi:lohi + 1],
                                op0=AL.mult, op1=AL.add)
                    if s == n_s // 2 - 1:
                        # first-half abs-max starts while second half runs
                        nc.vector.tensor_reduce(
                            out=rmax_a, in_=y2[:, :, 0:LOHI // 2],
                            axis=mybir.AxisListType.X, op=AL.max,
                            apply_absolute_value=True)
                # ---- gamma ----
                nc.vector.tensor_reduce(
                    out=rmax, in_=y2[:, :, LOHI // 2:],
                    axis=mybir.AxisListType.X, op=AL.max,
                    apply_absolute_value=True)
                nc.vector.tensor_tensor(out=rmax, in0=rmax, in1=rmax_a,
                                        op=AL.max)
                rmT = smallp[:, 0:t_blk]
                nc.tensor.transpose(rmT[0:t_blk, :], rmax, id128)
                gam = gp.tile([t_blk, 1], F32, tag="gam")
                nc.vector.tensor_reduce(
                    out=gam, in_=rmT[0:t_blk, :],
                    axis=mybir.AxisListType.X, op=AL.max)
                nc.vector.tensor_scalar_add(out=gam, in0=gam, scalar1=EPS)
                so = gp.tile([t_blk, 1], F32, tag="so")
                nc.vector.tensor_tensor(out=so, in0=gam, in1=ws7, op=AL.mult)
                sbc = gp.tile([t_blk, 1], F32, tag="sbc")
                nc.vector.reciprocal(out=sbc, in_=gam)
                nc.vector.tensor_scalar_mul(out=sbc, in0=sbc, scalar1=ACT_QB)
                # broadcast sbc over partitions via PE
                sbT = smallp[0:1, 128:128 + t_blk]
                nc.tensor.transpose(sbT, sbc, id128)
                sbrow = gp.tile([1, t_blk], F32, tag="sbrow")
                nc.scalar.copy(sbrow, sbT)
                sbb = smallp[:, 256:256 + t_blk]
                nc.tensor.matmul(sbb, lhsT=onesrow, rhs=sbrow,
                                 start=True, stop=True)
                sbbS = gp.tile([128, t_blk], F32, tag="sbbs")
                nc.vector.tensor_copy(out=sbbS, in_=sbb)
                # ---- quant (Pool): y2 *= sbb ; round; cast fp8 ----
                # two h-halves so the main matmul can start on early chunks
                qb = qp.tile([128, CH, t_blk], FP8, tag="q")
                for qh in range(2):
                    hsl = slice(qh * (LOHI // 2), (qh + 1) * (LOHI // 2))
                    a1, a2 = bass.broadcast_tensor_aps(
                        y2[:, :, hsl],
                        sbbS.rearrange("p (t o) -> p t o", o=1))
                    nc.gpsimd.tensor_tensor(out=y2[:, :, hsl], in0=a1,
                                            in1=a2, op=AL.mult)
                    nc.gpsimd.tensor_scalar(
                        out=qb[:, hsl, :].rearrange("p c t -> p t c"),
                        in0=y2[:, :, hsl], scalar1=MAGIC,
                        scalar2=MAGIC, op0=AL.add, op1=AL.subtract)
                # ---- main matmul: fp8 DoubleRow, q stationary, 4 banks ----
                ot = op_.tile([128, osh], out_dt, tag="ot")
                psums = [psm.tile([128, obw], F32, tag=f"pm{ob}",
                                  name=f"pm{ob}")
                         for ob in range(n_ob)]
                for k2 in range(CH // 2):
                    lhs = qb[:, 2 * k2:2 * k2 + 2, :]
                    for ob in range(n_ob):
                        nc.tensor.matmul(
                            psums[ob], lhsT=lhs,
                            rhs=wq2[k2][:, :, ob * obw:(ob + 1) * obw],
                            start=(k2 == 0), stop=(k2 == CH // 2 - 1),
                            perf_mode=DR)
                # evac split across Act and DVE (scale = gamma*wscale/7)
                for ob in range(2):
                    nc.scalar.activation(
                        out=ot[:, ob * obw:(ob + 1) * obw],
                        in_=psums[ob], func=AF.Copy, scale=so)
                for ob in range(2, n_ob):
                    nc.vector.tensor_scalar_mul(
                        out=ot[:, ob * obw:(ob + 1) * obw],
                        in0=psums[ob], scalar1=so)
                nc.sync.dma_start(out=out_d[t0:t0 + t_blk, :], in_=ot)

    return nc


def _split_multi_waits(nc):
    """walrus's CTRL encoder fits one sem-wait per instruction; Tile can emit
    several (e.g. the kernel-tail drain). Hoist extras onto standalone
    InstEventSemaphore carriers inserted just before the instruction."""
    import copy

    m = nc.m
    new_module = copy.replace(m, functions=[])
    ctr = 0
    for function in m.functions:
        new_function = copy.replace(function, blocks=[])
        new_function.set_allocations_from_list(function.allocations)
        for block in function.blocks:
            new_insts = []
            for inst in block.instructions:
                si = inst.sync_info
                ow = list(si.on_wait) if si is not None and si.on_wait else []
                if len(ow) > 1:
                    for w in ow[:-1]:
                        ctr += 1
                        new_insts.append(mybir.InstEventSemaphore(
                            name=f"I-wsplit-{ctr}",
                            engine=inst.engine,
                            ins=[], outs=[],
                            sync_info=mybir.SyncInfo(on_wait=[w],
                                                     on_update=[])))
                    inst = copy.replace(
                        inst,
                        sync_info=mybir.SyncInfo(on_wait=[ow[-1]],
                                                 on_update=si.on_update))
                new_insts.append(inst)
            new_block = copy.replace(block, instructions=new_insts)
            new_function.blocks.append(new_block)
        new_module.functions.append(new_function)
    nc.m = new_module
    return ctr


def host_prep(x, weight, had_scale, had_shift, n_cores=N_CORES, osh=None):
    """Shard + re-layout inputs for the SPMD program. Layout prep only."""
    T = int(np.prod(x.shape[:-1]))
    osh = osh or weight.shape[0] // n_cores
    H128p, S2, perm = host_consts()
    # pre-tile x to the device block layout [blk, p, c, t] (t_blk=128)
    t_blk = 128
    x4 = x.reshape(T // t_blk, t_blk, CH, 128)      # [blk, t, c, p]
    xt = np.ascontiguousarray(x4.transpose(0, 3, 2, 1))  # [blk, p, c, t]
    id128 = np.eye(128, dtype=np.float32)
    # fold the 1/64 FWHT normalization into the had_scale operand
    hs2 = np.ascontiguousarray(
        (had_scale[perm] / 64.0).reshape(CH, 128).T)  # [128(p2), 32(kk)]
    hb2 = np.ascontiguousarray(had_shift[perm].reshape(CH, 128).T)
    in_maps = []
    for core in range(n_cores):
        wsh = weight[core * osh:(core + 1) * osh, :]  # [osh, IN]
        wt = np.ascontiguousarray(wsh[:, perm].T)     # [IN(perm j), osh]
        in_maps.append({
            "xt": xt, "wt": wt, "hs2": hs2, "hb2": hb2,
            "h128p": H128p, "s2": S2, "id128": id128,
        })
    return in_maps


_PROGRAM_CACHE = {}


def _get_program(key, **kwargs):
    if key not in _PROGRAM_CACHE:
        nc = build_program(**kwargs)
        _split_multi_waits(nc)
        _PROGRAM_CACHE[key] = nc
    return _PROGRAM_CACHE[key]


def _timed_pjrt(nc, in_maps, n_cores, ks=(1, 17), reps=3):
    """Run via PJRT shard_map with device-resident inputs.

    Timing: dispatches pipeline asynchronously, so wall(K executions) =
    dispatch_overhead + K * device_time. The slope between two batch sizes
    isolates the marginal device execution time from the (~75ms here)
    axon-tunnel dispatch latency. Returns (results, times_dict).
    """
    import time

    import jax
    import jax.numpy as jnp
    from jax.sharding import Mesh, NamedSharding, PartitionSpec
    from jax.experimental.shard_map import shard_map

    from concourse import bass2jax, mybir as mb

    bass2jax.install_neuronx_cc_hook()

    partition_name = (nc.partition_id_tensor.name
                      if nc.partition_id_tensor else None)
    in_names, out_names, out_avals, zero_outs = [], [], [], []
    for alloc in nc.m.functions[0].allocations:
        if not isinstance(alloc, mb.MemoryLocationSet):
            continue
        name = alloc.memorylocations[0].name
        if alloc.kind == "ExternalInput":
            if name != partition_name:
                in_names.append(name)
        elif alloc.kind == "ExternalOutput":
            out_names.append(name)
            shape = tuple(alloc.tensor_shape)
            dtype = mb.dt.np(alloc.dtype)
            out_avals.append(jax.core.ShapedArray(shape, dtype))
            zero_outs.append(np.zeros(shape, dtype))
    n_params = len(in_names)
    all_in_names = list(in_names) + list(out_names)
    if partition_name is not None:
        all_in_names.append(partition_name)

    def _body(*args):
        operands = list(args)
        if partition_name is not None:
            operands.append(bass2jax.partition_id_tensor())
        outs = bass2jax._bass_exec_p.bind(
            *operands,
            out_avals=tuple(out_avals),
            in_names=tuple(all_in_names),
            out_names=tuple(out_names),
            lowering_input_output_aliases=(),
            sim_require_finite=True,
            sim_require_nnan=True,
            nc=nc,
        )
        return tuple(outs)

    devices = jax.devices()[:n_cores]
    mesh = Mesh(np.asarray(devices), ("core",))
    spec = NamedSharding(mesh, PartitionSpec("core"))
    n_outs = len(out_names)
    donate = tuple(range(n_params, n_params + n_outs))
    sharded = jax.jit(
        shard_map(_body, mesh=mesh,
                  in_specs=(PartitionSpec("core"),) * (n_params + n_outs),
                  out_specs=(PartitionSpec("core"),) * n_outs,
                  check_rep=False),
        donate_argnums=donate, keep_unused=True)

    concat_in = [
        np.concatenate([np.asarray(in_maps[c][nm]) for c in range(n_cores)],
                       axis=0)
        for nm in in_names
    ]
    dev_in = [jax.device_put(a, spec) for a in concat_in]
    zero_shapes = [(n_cores * z.shape[0], *z.shape[1:]) for z in zero_outs]
    make_zeros = jax.jit(
        lambda: tuple(jnp.zeros(s, z.dtype)
                      for s, z in zip(zero_shapes, zero_outs)),
        out_shardings=(spec,) * n_outs)

    out = sharded(*dev_in, *make_zeros())  # warmup/compile
    jax.block_until_ready(out)

    times = {}
    for k in ks:
        times[k] = []
        for _ in range(reps):
            zs = [make_zeros() for _ in range(k)]
            jax.block_until_ready(zs)
            t0 = time.perf_counter()
            for i in range(k):
                out = sharded(*dev_in, *zs[i])
            jax.block_until_ready(out)
            times[k].append(time.perf_counter() - t0)
    results = [
        {nm: np.asarray(out[i]).reshape(n_cores, *out_avals[i].shape)[c]
         for i, nm in enumerate(out_names)}
        for c in range(n_cores)
    ]
    return results, times


def run(x, weight, had_scale, had_shift, trace=False, **trace_kwargs):
    x = np.asarray(x, dtype=np.float32)
    weight = np.asarray(weight, dtype=np.float32)
    had_scale = np.asarray(had_scale, dtype=np.float32)
    had_shift = np.asarray(had_shift, dtype=np.float32)
    batch_shape = x.shape[:-1]
    T = int(np.prod(batch_shape))
    nc = _get_program(("full", T), T=T)
    in_maps = host_prep(x, weight, had_scale, had_shift)
    if trace:
        results, times = _timed_pjrt(nc, in_maps, N_CORES)
    else:
        res = run_bass_kernel_spmd(nc, in_maps, core_ids=list(range(N_CORES)))
        results, times = res.results, None
    shards = [np.asarray(results[c]["out"], dtype=np.float32)
              for c in range(N_CORES)]
    out = np.concatenate(shards, axis=1).reshape(*batch_shape, OUT)
    return out, times


def kernel(**inputs):
    out, _ = run(inputs["x"], inputs["weight"], inputs["had_scale"],
                 inputs["had_shift"])
    return out


if __name__ == "__main__":
    nc = build_program(T=256)
    print("build ok")
